# revision 12
# baseline (speedup 1.0000x reference)
"""Expert-parallel grouped-GEMM FFN (MoE expert module) for TRN2, 8 NeuronCores.

Problem: xs [16384, 1024] grouped contiguously into 16 experts x 1024 tokens.
Per expert e: y = relu(x @ w1[e].T + b1[e]) @ w2[e].T + b2[e].

Sharding: expert-parallel, 2 experts per core. Each core computes its two
experts' FFN independently; outputs are disjoint row-blocks of the result, so
no collectives are needed.

Precision: weights and activations are bf16 (host-side cast), accumulation and
biases fp32.  l2 relative error ~3e-3, well inside the 2e-2 gate.

The matmul stream itself runs at the N=512 issue floor (~213 ns/MM warm), so
v2 attacks the only remaining overheads seen in the trace:
  - Startup: the framework preamble blocks all engine queues until ~7.2us, so
    DMA data cannot land before ~10.5us; the old 80-matmul warmup padded the
    PE queue until 13.9us (NX-paced ~91ns each, in-order ahead of the real
    chains).  v2 trims the warmup to ~26 MMs so the first real chain issues
    right as its x/w1 deps land, with the warmup still covering the HAM
    clock-gate ramp (~3.4us of busy to reach 2.4 GHz).
  - Periodic notification stalls: the profile shows a 432 ns PE stall every
    10.79 us - the notification buffer (~640 entries) draining; DMA packets
    (one per partition row) dominate the notification rate.  v2 batches DMAs
    into wider per-partition rows: w1 in 4-k-tile groups (8KB rows), w2 in
    2-dd groups (16KB rows), y per (expert, token-half) (8KB rows), x as one
    2MB tile per expert (16KB rows), b1+b2 combined - cutting packets from
    ~17.6k to ~6k and the stall count proportionally.
  - Tail: the last d-tile runs as two N=256 column-half chains in separate
    PSUM banks so the post-final-matmul drainage covers only 256 columns
    (unchanged from v1).

Per-core schedule (per expert, all matmuls N=512, 128-contraction):
  - mm1: for each of 32 h-tiles k, two 8-matmul PSUM chains (contraction D),
    ACT evicts relu(acc + b1) -> h[k] bf16 in SBUF.  The second token-half
    chain trails DEFER k-tiles so expert 0's first chains need only the first
    token-half of x.
  - mm2: for each of 8 d-tiles, two interleaved 32-matmul PSUM chains
    (contraction H); ACT evicts acc + b2 into a per-token-half y tile that is
    written back in one DMA per (expert, token-half).
  - Startup-critical transfers ride one sync-ring in priority order (w1[0]
    single k-tiles with the x halves behind w1[0,0], then x-nt1, then 4-k
    w1 groups); w2 groups are issued mid-mm1 so the 2MB transfers hide under
    the trailing mm1 chains.
"""

import numpy as np
import ml_dtypes

import concourse.bacc as bacc
import concourse.mybir as mybir
import concourse.tile as tile
from concourse.bass_utils import run_bass_kernel_spmd

P = 128                 # SBUF partitions / PE array dim
D = 1024                # model dim
H = 4096                # hidden dim
E = 16                  # experts
N_TOK = 16384           # total tokens
N_CORES = 8
E_LOC = E // N_CORES    # experts per core = 2
NE = N_TOK // E         # tokens per expert = 1024
DC = D // P             # 8  (d chunks: mm1 contraction / mm2 output)
HC = H // P             # 32 (h chunks)
NT = 512                # matmul moving free dim (one PSUM bank of fp32)
NN = NE // NT           # 2  (token tiles per expert)
DEFER = 6               # k-tiles the second token-half chain trails by
W1G = 4                 # k-tiles per batched w1 DMA (8KB dram rows)
NG1 = HC // W1G         # 8 w1 groups
HEAD = 4                # expert 0's first k-tiles ride as single-k DMAs
W2G = 2                 # dd-tiles per batched w2 DMA (16KB dram rows)
NG2 = DC // W2G         # 4 w2 groups
WARM_MM = 40            # HAM warmup matmuls (cover preamble->first-DMA-landing)

F32 = mybir.dt.float32
BF16 = mybir.dt.bfloat16
NP_BF16 = ml_dtypes.bfloat16

_CACHE = {}


def _build_nc():
    nc = bacc.Bacc(None, target_bir_lowering=False)

    # Host-tiled layouts (see _prep_in_maps for the exact index maps):
    #   xt  [s, nt, p, c, j]      = x_e[nt*512+j, c*128+p]          (8KB rows)
    #   w1t [s, g, p, u, c, j]    = w1[e, (g*4+u)*128+j, c*128+p]   (8KB rows)
    #   w2t [s, g, p, v, k, j]    = w2[e, (g*2+v)*128+j, k*128+p]   (16KB rows)
    #   br  [s, p, 0:HC]          = b1[e, k*128+p]
    #   br  [s, p, HC+dd]         = b2[e, dd*128+p]
    #   yt  [s, nt, p, dd, j]     = y_e[nt*512+j, dd*128+p]         (8KB rows)
    xt = nc.dram_tensor("xt", [E_LOC, NN, P, DC, NT], BF16, kind="ExternalInput")
    w1t = nc.dram_tensor("w1t", [E_LOC, NG1, P, W1G, DC, P], BF16,
                         kind="ExternalInput")
    w2t = nc.dram_tensor("w2t", [E_LOC, NG2, P, W2G, HC, P], BF16,
                         kind="ExternalInput")
    br = nc.dram_tensor("br", [E_LOC, P, HC + DC], F32, kind="ExternalInput")
    yt = nc.dram_tensor("yt", [E_LOC, NN, P, DC, NT], BF16, kind="ExternalOutput")

    with tile.TileContext(nc) as tc:
        with (
            tc.tile_pool(name="xpool", bufs=NN) as xpool,
            tc.tile_pool(name="hpool", bufs=HC + 4) as hpool,
            tc.tile_pool(name="w1hpool", bufs=HEAD) as w1hpool,
            tc.tile_pool(name="w1pool", bufs=4) as w1pool,
            tc.tile_pool(name="w2pool", bufs=2) as w2pool,
            tc.tile_pool(name="ybpool", bufs=NN) as ybpool,
            tc.tile_pool(name="ypool", bufs=4) as ypool,
            tc.tile_pool(name="cpool", bufs=E_LOC) as cpool,
            tc.tile_pool(name="wpool", bufs=1) as wpool,
            tc.tile_pool(name="ps1", bufs=4, space="PSUM") as ps1,
            tc.tile_pool(name="ps2", bufs=4, space="PSUM") as ps2,
        ):
            # HAM warm-up: throwaway matmuls on a zeroed scratch tile keep the
            # PE busy from preamble-exit (~7.2us) until the first chain's x/w1
            # DMAs land (~10.5us), so the clock gate reaches 8/8 (2.4 GHz)
            # before the real chains start.  These are in-order ahead of the
            # real chains (NX-paced ~107ns each cold), so the count must NOT
            # overshoot the DMA landing time.
            warm = wpool.tile([P, P], BF16)
            nc.gpsimd.memset(warm[:], 0.0)
            warm_acc = ps1.tile([P, NT], F32, name="acc", tag="acc")
            for _ in range(WARM_MM):
                nc.tensor.matmul(warm_acc[:, 0:P], warm[:], warm[:],
                                 start=True, stop=True)

            # Expert 0's first x token-half rides the sync ring right behind
            # the first w1 k-tile: HBM is the startup bottleneck, and the
            # first chain's deps must complete earliest.  Tile dependencies
            # are tracked per tile (a reader waits for ALL of a tile's
            # writers), so every startup-critical transfer gets its own tile:
            # per-k w1 head tiles and per-token-half x tiles.
            x_cur = [xpool.tile([P, DC, NT], BF16, name="x_t", tag="x_t")
                     for _ in range(NN)]

            for s in range(E_LOC):
                # ---------------- mm1: h = relu(x @ w1.T + b1) ----------------
                h_tiles = []
                w1h_tiles = []
                w1g_tiles = {}
                w2g_tiles = []
                n_head = HEAD if s == 0 else 0

                def load_w1_group(g, s=s):
                    t = w1pool.tile([P, W1G, DC, P], BF16, name="w1g", tag="w1g")
                    nc.sync.dma_start(out=t[:], in_=w1t[s, g])
                    w1g_tiles[g] = t

                def w1_slice(k, c, s=s, n_head=n_head):
                    if k < n_head:
                        return w1h_tiles[k][:, c, :]
                    return w1g_tiles[k // W1G][:, k % W1G, c, :]

                def mm1_chain(k, nt, s=s):
                    acc = ps1.tile([P, NT], F32, name="acc", tag="acc")
                    for c in range(DC):
                        nc.tensor.matmul(
                            acc[:],
                            w1_slice(k, c),
                            x_cur[nt][:, c, :],
                            start=(c == 0),
                            stop=(c == DC - 1),
                        )
                    nc.scalar.activation(
                        h_tiles[k][:, nt * NT : (nt + 1) * NT],
                        acc[:],
                        mybir.ActivationFunctionType.Relu,
                        bias=b_t[:, k : k + 1],
                    )

                if s == 0:
                    # Startup is DMA-landing-bound: per-queue transfers run
                    # one at a time at a row-size-dependent rate (2KB rows
                    # ~175 GB/s, 8KB rows ~390 GB/s), so the first chain's
                    # deps are spread across two engine rings in parallel:
                    #   sync:   x-nt0 (1MB, 8KB rows), w1[2], w1[3],
                    #           w1 group1, x-nt1, groups 2..7
                    #   gpsimd: w1[0], w1[1] single k-tiles
                    #   scalar: b1/b2 (its DMA ring first-fetch lags ~2-3us)
                    # First chain (needs w1[0] + x-nt0) is ready ~12us.
                    nc.sync.dma_start(out=x_cur[0][:], in_=xt[0, 0])
                    for k in range(HEAD):
                        t = w1hpool.tile([P, DC, P], BF16,
                                         name="w1h", tag="w1h")
                        eng = nc.gpsimd if k < 2 else nc.sync
                        eng.dma_start(out=t[:], in_=w1t[0, k // W1G, :,
                                                        k % W1G])
                        w1h_tiles.append(t)
                    b_t = cpool.tile([P, HC + DC], F32)
                    nc.scalar.dma_start(out=b_t[:], in_=br[s])
                    load_w1_group(1)
                    nc.sync.dma_start(out=x_cur[1][:], in_=xt[0, 1])
                else:
                    b_t = cpool.tile([P, HC + DC], F32)
                    nc.scalar.dma_start(out=b_t[:], in_=br[s])
                    load_w1_group(0)
                    load_w1_group(1)

                for k in range(HC):
                    # Keep ~2 w1 groups of lookahead on the sync ring.
                    if k % W1G == 0:
                        g = k // W1G + 2
                        if g < NG1 and (g >= n_head // W1G):
                            load_w1_group(g)
                    # w2 group DMAs (2MB each) issue mid-mm1 so they hide
                    # under the trailing mm1 chains.
                    if k == 20 or k == 28:
                        t = w2pool.tile([P, W2G, HC, P], BF16,
                                        name="w2g", tag="w2g")
                        nc.sync.dma_start(out=t[:], in_=w2t[s, len(w2g_tiles)])
                        w2g_tiles.append(t)
                    h_tiles.append(hpool.tile([P, NE], BF16, name="h_t", tag="h_t"))
                    mm1_chain(k, 0)
                    if k >= DEFER:
                        mm1_chain(k - DEFER, 1)
                for k in range(HC - DEFER, HC):
                    mm1_chain(k, 1)

                # Prefetch next expert's x while this expert's mm2 runs
                # (xpool bufs=NN defers it until mm1(s) retires).
                if s + 1 < E_LOC:
                    x_next = [xpool.tile([P, DC, NT], BF16,
                                         name="x_t", tag="x_t")
                              for _ in range(NN)]
                    for nt in range(NN):
                        nc.scalar.dma_start(out=x_next[nt][:],
                                            in_=xt[s + 1, nt])
                    x_cur = x_next

                # ---------------- mm2: y = h @ w2.T + b2 ----------------
                # Evictions land in one bf16 y tile per token-half; each tile
                # is written back in a single 8KB-row DMA.  For the last
                # expert the final d-tile stays fine-grained (N=256 halves)
                # to keep the kernel tail short.
                y_big = [ybpool.tile([P, DC, NT], BF16, name="y_b", tag="y_b")
                         for _ in range(NN)]
                DD_BATCH = DC - 1 if s == E_LOC - 1 else DC

                for dd in range(DC):
                    if dd >= len(w2g_tiles) * W2G:
                        t = w2pool.tile([P, W2G, HC, P], BF16,
                                        name="w2g", tag="w2g")
                        nc.sync.dma_start(out=t[:], in_=w2t[s, len(w2g_tiles)])
                        w2g_tiles.append(t)
                    w2_t = w2g_tiles[dd // W2G]
                    v = dd % W2G
                    last = s == E_LOC - 1 and dd == DC - 1
                    acc2s = [ps2.tile([P, NT], F32, name="acc2", tag="acc2")
                             for _ in range(NN)]

                    if last:
                        # nt0: plain chain; its eviction + writeback hide under
                        # nt1's work.  nt1 runs as two N=256 column-half chains
                        # in separate PSUM banks: the first half's eviction and
                        # writeback hide under the second half's ~3.5us chain,
                        # and the post-final-matmul drainage (evict + DMA +
                        # completion receipt) covers only 256 columns.
                        for k in range(HC):
                            nc.tensor.matmul(
                                acc2s[0][:],
                                w2_t[:, v, k, :],
                                h_tiles[k][:, 0:NT],
                                start=(k == 0),
                                stop=(k == HC - 1),
                            )
                        y_tile = ypool.tile([P, NT], BF16)
                        nc.scalar.activation(
                            y_tile[:],
                            acc2s[0][:],
                            mybir.ActivationFunctionType.Identity,
                            bias=b_t[:, HC + dd : HC + dd + 1],
                        )
                        nc.scalar.dma_start(
                            out=yt[s, 0, :, dd, :],
                            in_=y_tile[:],
                        )
                        HN = NT // 2
                        accR = ps2.tile([P, NT], F32, name="acc2", tag="acc2")
                        for half, acch in ((0, acc2s[1]), (1, accR)):
                            lo = half * HN
                            for k in range(HC):
                                nc.tensor.matmul(
                                    acch[:, 0:HN],
                                    w2_t[:, v, k, :],
                                    h_tiles[k][:, NT + lo : NT + lo + HN],
                                    start=(k == 0),
                                    stop=(k == HC - 1),
                                )
                            y_half = ypool.tile([P, NT], BF16)
                            nc.scalar.activation(
                                y_half[:, 0:HN],
                                acch[:, 0:HN],
                                mybir.ActivationFunctionType.Identity,
                                bias=b_t[:, HC + dd : HC + dd + 1],
                            )
                            nc.scalar.dma_start(
                                out=yt[s, 1, :, dd, lo : lo + HN],
                                in_=y_half[:, 0:HN],
                            )
                    else:
                        for k in range(HC):
                            for nt in range(NN):
                                nc.tensor.matmul(
                                    acc2s[nt][:],
                                    w2_t[:, v, k, :],
                                    h_tiles[k][:, nt * NT : (nt + 1) * NT],
                                    start=(k == 0),
                                    stop=(k == HC - 1),
                                )
                        for nt in range(NN):
                            nc.scalar.activation(
                                y_big[nt][:, dd, :],
                                acc2s[nt][:],
                                mybir.ActivationFunctionType.Identity,
                                bias=b_t[:, HC + dd : HC + dd + 1],
                            )
                        if dd == DD_BATCH - 1:
                            for nt in range(NN):
                                nc.scalar.dma_start(
                                    out=yt[s, nt, :, 0:DD_BATCH, :],
                                    in_=y_big[nt][:, 0:DD_BATCH, :],
                                )

    nc.finalize()
    return nc


def _get_nc():
    if "nc" not in _CACHE:
        _CACHE["nc"] = _build_nc()
    return _CACHE["nc"]


def _prep_in_maps(xs, w1, b1, w2, b2):
    xs = np.asarray(xs, dtype=np.float32).astype(NP_BF16)
    w1 = np.asarray(w1, dtype=np.float32).astype(NP_BF16)
    b1 = np.asarray(b1, dtype=np.float32)
    w2 = np.asarray(w2, dtype=np.float32).astype(NP_BF16)
    b2 = np.asarray(b2, dtype=np.float32)

    x3 = xs.reshape(E, NE, D)
    in_maps = []
    for core in range(N_CORES):
        es = [E_LOC * core + s for s in range(E_LOC)]
        # xt[s, nt, p, c, j] = x_e[nt*512+j, c*128+p]
        xt = np.stack(
            [x3[e].reshape(NN, NT, DC, P).transpose(0, 3, 2, 1) for e in es]
        )
        # w1t[s, g, p, u, c, j] = w1[e, (g*4+u)*128+j, c*128+p]
        w1t = np.stack(
            [w1[e].reshape(NG1, W1G, P, DC, P).transpose(0, 4, 1, 3, 2)
             for e in es]
        )
        # w2t[s, g, p, v, k, j] = w2[e, (g*2+v)*128+j, k*128+p]
        w2t = np.stack(
            [w2[e].reshape(NG2, W2G, P, HC, P).transpose(0, 4, 1, 3, 2)
             for e in es]
        )
        # br[s, p, k] = b1[e, k*128+p];  br[s, p, HC+dd] = b2[e, dd*128+p]
        brm = np.stack(
            [np.concatenate(
                [b1[e].reshape(HC, P).T, b2[e].reshape(DC, P).T], axis=1)
             for e in es]
        )
        in_maps.append(
            {
                "xt": np.ascontiguousarray(xt),
                "w1t": np.ascontiguousarray(w1t),
                "w2t": np.ascontiguousarray(w2t),
                "br": np.ascontiguousarray(brm),
            }
        )
    return in_maps


def _gather(results):
    y = np.empty((N_TOK, D), dtype=np.float32)
    for core in range(N_CORES):
        out = results[core]["yt"]  # [E_LOC, NN, P, DC, NT] bf16
        for s in range(E_LOC):
            e = E_LOC * core + s
            # yt[s, nt, p, dd, j] = y_e[nt*512+j, dd*128+p]
            for nt in range(NN):
                y[e * NE + nt * NT : e * NE + (nt + 1) * NT] = (
                    out[s, nt].transpose(2, 1, 0).reshape(NT, D)
                    .astype(np.float32)
                )
    return y


def _run(in_maps, **kwargs):
    nc = _get_nc()
    return run_bass_kernel_spmd(nc, in_maps, core_ids=list(range(N_CORES)), **kwargs)


def kernel(xs, fwd_expert_count, w1, b1, w2, b2):
    # fwd_expert_count is uniform (N_TOK // E per expert) by construction,
    # matching the reference, which also hardcodes the uniform grouping.
    in_maps = _prep_in_maps(xs, w1, b1, w2, b2)
    res = _run(in_maps)
    return _gather(res.results)


# revision 14
# speedup vs baseline: 1.0004x; 1.0004x over previous
"""Expert-parallel grouped-GEMM FFN (MoE expert module) for TRN2, 8 NeuronCores.

Problem: xs [16384, 1024] grouped contiguously into 16 experts x 1024 tokens.
Per expert e: y = relu(x @ w1[e].T + b1[e]) @ w2[e].T + b2[e].

Sharding: expert-parallel, 2 experts per core. Each core computes its two
experts' FFN independently; outputs are disjoint row-blocks of the result, so
no collectives are needed.

Precision: weights and activations are bf16 (host-side cast), accumulation and
biases fp32.  l2 relative error ~3e-3, well inside the 2e-2 gate.

The matmul stream itself runs at the N=512 issue floor (~213 ns/MM warm), so
v2 attacks the only remaining overheads seen in the trace:
  - Startup: the framework preamble blocks all engine queues until ~7.2us, so
    DMA data cannot land before ~10.5us; the old 80-matmul warmup padded the
    PE queue until 13.9us (NX-paced ~91ns each, in-order ahead of the real
    chains).  v2 trims the warmup to ~26 MMs so the first real chain issues
    right as its x/w1 deps land, with the warmup still covering the HAM
    clock-gate ramp (~3.4us of busy to reach 2.4 GHz).
  - Periodic notification stalls: the profile shows a 432 ns PE stall every
    10.79 us - the notification buffer (~640 entries) draining; DMA packets
    (one per partition row) dominate the notification rate.  v2 batches DMAs
    into wider per-partition rows: w1 in 4-k-tile groups (8KB rows), w2 in
    2-dd groups (16KB rows), y per (expert, token-half) (8KB rows), x as one
    2MB tile per expert (16KB rows), b1+b2 combined - cutting packets from
    ~17.6k to ~6k and the stall count proportionally.
  - Tail: the last d-tile runs as two N=256 column-half chains in separate
    PSUM banks so the post-final-matmul drainage covers only 256 columns
    (unchanged from v1).

Per-core schedule (per expert, all matmuls N=512, 128-contraction):
  - mm1: for each of 32 h-tiles k, two 8-matmul PSUM chains (contraction D),
    ACT evicts relu(acc + b1) -> h[k] bf16 in SBUF.  The second token-half
    chain trails DEFER k-tiles so expert 0's first chains need only the first
    token-half of x.
  - mm2: for each of 8 d-tiles, two interleaved 32-matmul PSUM chains
    (contraction H); ACT evicts acc + b2 into a per-token-half y tile that is
    written back in one DMA per (expert, token-half).
  - Startup-critical transfers ride one sync-ring in priority order (w1[0]
    single k-tiles with the x halves behind w1[0,0], then x-nt1, then 4-k
    w1 groups); w2 groups are issued mid-mm1 so the 2MB transfers hide under
    the trailing mm1 chains.
"""

import numpy as np
import ml_dtypes

import concourse.bacc as bacc
import concourse.mybir as mybir
import concourse.tile as tile
from concourse.bass_utils import run_bass_kernel_spmd

P = 128                 # SBUF partitions / PE array dim
D = 1024                # model dim
H = 4096                # hidden dim
E = 16                  # experts
N_TOK = 16384           # total tokens
N_CORES = 8
E_LOC = E // N_CORES    # experts per core = 2
NE = N_TOK // E         # tokens per expert = 1024
DC = D // P             # 8  (d chunks: mm1 contraction / mm2 output)
HC = H // P             # 32 (h chunks)
NT = 512                # matmul moving free dim (one PSUM bank of fp32)
NN = NE // NT           # 2  (token tiles per expert)
DEFER = 6               # k-tiles the second token-half chain trails by
W1G = 4                 # k-tiles per batched w1 DMA (8KB dram rows)
NG1 = HC // W1G         # 8 w1 groups
HEAD = 4                # expert 0's first k-tiles ride as single-k DMAs
W2G = 2                 # dd-tiles per batched w2 DMA (16KB dram rows)
NG2 = DC // W2G         # 4 w2 groups
WARM_MM = 50            # HAM warmup matmuls (cover preamble->first-DMA-landing)

F32 = mybir.dt.float32
BF16 = mybir.dt.bfloat16
NP_BF16 = ml_dtypes.bfloat16

_CACHE = {}


def _build_nc():
    nc = bacc.Bacc(None, target_bir_lowering=False)

    # Host-tiled layouts (see _prep_in_maps for the exact index maps):
    #   xt  [s, nt, p, c, j]      = x_e[nt*512+j, c*128+p]          (8KB rows)
    #   w1t [s, g, p, u, c, j]    = w1[e, (g*4+u)*128+j, c*128+p]   (8KB rows)
    #   w2t [s, g, p, v, k, j]    = w2[e, (g*2+v)*128+j, k*128+p]   (16KB rows)
    #   br  [s, p, 0:HC]          = b1[e, k*128+p]
    #   br  [s, p, HC+dd]         = b2[e, dd*128+p]
    #   yt  [s, nt, p, dd, j]     = y_e[nt*512+j, dd*128+p]         (8KB rows)
    xt = nc.dram_tensor("xt", [E_LOC, NN, P, DC, NT], BF16, kind="ExternalInput")
    w1t = nc.dram_tensor("w1t", [E_LOC, NG1, P, W1G, DC, P], BF16,
                         kind="ExternalInput")
    w2t = nc.dram_tensor("w2t", [E_LOC, NG2, P, W2G, HC, P], BF16,
                         kind="ExternalInput")
    br = nc.dram_tensor("br", [E_LOC, P, HC + DC], F32, kind="ExternalInput")
    yt = nc.dram_tensor("yt", [E_LOC, NN, P, DC, NT], BF16, kind="ExternalOutput")

    with tile.TileContext(nc) as tc:
        with (
            tc.tile_pool(name="xpool", bufs=NN) as xpool,
            tc.tile_pool(name="hpool", bufs=HC + 4) as hpool,
            tc.tile_pool(name="w1hpool", bufs=HEAD) as w1hpool,
            tc.tile_pool(name="w1pool", bufs=4) as w1pool,
            tc.tile_pool(name="w2pool", bufs=2) as w2pool,
            tc.tile_pool(name="ybpool", bufs=NN) as ybpool,
            tc.tile_pool(name="ypool", bufs=4) as ypool,
            tc.tile_pool(name="cpool", bufs=E_LOC) as cpool,
            tc.tile_pool(name="wpool", bufs=1) as wpool,
            tc.tile_pool(name="ps1", bufs=4, space="PSUM") as ps1,
            tc.tile_pool(name="ps2", bufs=4, space="PSUM") as ps2,
        ):
            # HAM warm-up: throwaway matmuls on a zeroed scratch tile keep the
            # PE busy from preamble-exit (~7.2us) until the first chain's x/w1
            # DMAs land (~10.5us), so the clock gate reaches 8/8 (2.4 GHz)
            # before the real chains start.  These are in-order ahead of the
            # real chains (NX-paced ~107ns each cold), so the count must NOT
            # overshoot the DMA landing time.
            warm = wpool.tile([P, P], BF16)
            nc.gpsimd.memset(warm[:], 0.0)
            warm_acc = ps1.tile([P, NT], F32, name="acc", tag="acc")
            for _ in range(WARM_MM):
                nc.tensor.matmul(warm_acc[:, 0:P], warm[:], warm[:],
                                 start=True, stop=True)

            # Expert 0's first x token-half rides the sync ring right behind
            # the first w1 k-tile: HBM is the startup bottleneck, and the
            # first chain's deps must complete earliest.  Tile dependencies
            # are tracked per tile (a reader waits for ALL of a tile's
            # writers), so every startup-critical transfer gets its own tile:
            # per-k w1 head tiles and per-token-half x tiles.
            x_cur = [xpool.tile([P, DC, NT], BF16, name="x_t", tag="x_t")
                     for _ in range(NN)]

            for s in range(E_LOC):
                # ---------------- mm1: h = relu(x @ w1.T + b1) ----------------
                h_tiles = []
                w1h_tiles = []
                w1g_tiles = {}
                w2g_tiles = []
                n_head = HEAD if s == 0 else 0

                def load_w1_group(g, s=s):
                    t = w1pool.tile([P, W1G, DC, P], BF16, name="w1g", tag="w1g")
                    nc.sync.dma_start(out=t[:], in_=w1t[s, g])
                    w1g_tiles[g] = t

                def w1_slice(k, c, s=s, n_head=n_head):
                    if k < n_head:
                        return w1h_tiles[k][:, c, :]
                    return w1g_tiles[k // W1G][:, k % W1G, c, :]

                def mm1_chain(k, nt, s=s):
                    acc = ps1.tile([P, NT], F32, name="acc", tag="acc")
                    for c in range(DC):
                        nc.tensor.matmul(
                            acc[:],
                            w1_slice(k, c),
                            x_cur[nt][:, c, :],
                            start=(c == 0),
                            stop=(c == DC - 1),
                        )
                    nc.scalar.activation(
                        h_tiles[k][:, nt * NT : (nt + 1) * NT],
                        acc[:],
                        mybir.ActivationFunctionType.Relu,
                        bias=b_t[:, k : k + 1],
                    )

                if s == 0:
                    # Startup is DMA-landing-bound, and only the sync ring's
                    # DGE starts promptly (secondary rings lag ~3.5us on
                    # their first fetch), so everything critical rides the
                    # sync ring serially, ordered by deadline.  Per-queue
                    # rate is row-size-dependent: 2KB rows ~175 GB/s, 8KB
                    # rows ~390 GB/s, so x-nt0 goes as one 8KB-row transfer.
                    # First chain (needs w1[0] + x-nt0) is ready ~13us.
                    w1h_tiles = [w1hpool.tile([P, DC, P], BF16,
                                              name="w1h", tag="w1h")
                                 for _ in range(HEAD)]
                    nc.sync.dma_start(out=w1h_tiles[0][:], in_=w1t[0, 0, :, 0])
                    nc.sync.dma_start(out=x_cur[0][:], in_=xt[0, 0])
                    for k in range(1, HEAD):
                        nc.sync.dma_start(out=w1h_tiles[k][:],
                                          in_=w1t[0, k // W1G, :, k % W1G])
                    b_t = cpool.tile([P, HC + DC], F32)
                    nc.scalar.dma_start(out=b_t[:], in_=br[s])
                    load_w1_group(1)
                    nc.sync.dma_start(out=x_cur[1][:], in_=xt[0, 1])
                else:
                    b_t = cpool.tile([P, HC + DC], F32)
                    nc.scalar.dma_start(out=b_t[:], in_=br[s])
                    load_w1_group(0)
                    load_w1_group(1)

                for k in range(HC):
                    # Keep ~2 w1 groups of lookahead on the sync ring.
                    if k % W1G == 0:
                        g = k // W1G + 2
                        if g < NG1 and (g >= n_head // W1G):
                            load_w1_group(g)
                    # w2 group DMAs (2MB each) issue mid-mm1 so they hide
                    # under the trailing mm1 chains.
                    if k == 20 or k == 28:
                        t = w2pool.tile([P, W2G, HC, P], BF16,
                                        name="w2g", tag="w2g")
                        nc.sync.dma_start(out=t[:], in_=w2t[s, len(w2g_tiles)])
                        w2g_tiles.append(t)
                    h_tiles.append(hpool.tile([P, NE], BF16, name="h_t", tag="h_t"))
                    mm1_chain(k, 0)
                    if k >= DEFER:
                        mm1_chain(k - DEFER, 1)
                for k in range(HC - DEFER, HC):
                    mm1_chain(k, 1)

                # Prefetch next expert's x while this expert's mm2 runs
                # (xpool bufs=NN defers it until mm1(s) retires).
                if s + 1 < E_LOC:
                    x_next = [xpool.tile([P, DC, NT], BF16,
                                         name="x_t", tag="x_t")
                              for _ in range(NN)]
                    for nt in range(NN):
                        nc.scalar.dma_start(out=x_next[nt][:],
                                            in_=xt[s + 1, nt])
                    x_cur = x_next

                # ---------------- mm2: y = h @ w2.T + b2 ----------------
                # Evictions land in one bf16 y tile per token-half; each tile
                # is written back in a single 8KB-row DMA.  For the last
                # expert the final d-tile stays fine-grained (N=256 halves)
                # to keep the kernel tail short.
                y_big = [ybpool.tile([P, DC, NT], BF16, name="y_b", tag="y_b")
                         for _ in range(NN)]
                DD_BATCH = DC - 1 if s == E_LOC - 1 else DC

                for dd in range(DC):
                    if dd >= len(w2g_tiles) * W2G:
                        t = w2pool.tile([P, W2G, HC, P], BF16,
                                        name="w2g", tag="w2g")
                        nc.sync.dma_start(out=t[:], in_=w2t[s, len(w2g_tiles)])
                        w2g_tiles.append(t)
                    w2_t = w2g_tiles[dd // W2G]
                    v = dd % W2G
                    last = s == E_LOC - 1 and dd == DC - 1
                    acc2s = [ps2.tile([P, NT], F32, name="acc2", tag="acc2")
                             for _ in range(NN)]

                    if last:
                        # nt0: plain chain; its eviction + writeback hide under
                        # nt1's work.  nt1 runs as two N=256 column-half chains
                        # in separate PSUM banks: the first half's eviction and
                        # writeback hide under the second half's ~3.5us chain,
                        # and the post-final-matmul drainage (evict + DMA +
                        # completion receipt) covers only 256 columns.
                        for k in range(HC):
                            nc.tensor.matmul(
                                acc2s[0][:],
                                w2_t[:, v, k, :],
                                h_tiles[k][:, 0:NT],
                                start=(k == 0),
                                stop=(k == HC - 1),
                            )
                        y_tile = ypool.tile([P, NT], BF16)
                        nc.scalar.activation(
                            y_tile[:],
                            acc2s[0][:],
                            mybir.ActivationFunctionType.Identity,
                            bias=b_t[:, HC + dd : HC + dd + 1],
                        )
                        nc.scalar.dma_start(
                            out=yt[s, 0, :, dd, :],
                            in_=y_tile[:],
                        )
                        HN = NT // 2
                        accR = ps2.tile([P, NT], F32, name="acc2", tag="acc2")
                        for half, acch in ((0, acc2s[1]), (1, accR)):
                            lo = half * HN
                            for k in range(HC):
                                nc.tensor.matmul(
                                    acch[:, 0:HN],
                                    w2_t[:, v, k, :],
                                    h_tiles[k][:, NT + lo : NT + lo + HN],
                                    start=(k == 0),
                                    stop=(k == HC - 1),
                                )
                            y_half = ypool.tile([P, NT], BF16)
                            nc.scalar.activation(
                                y_half[:, 0:HN],
                                acch[:, 0:HN],
                                mybir.ActivationFunctionType.Identity,
                                bias=b_t[:, HC + dd : HC + dd + 1],
                            )
                            nc.scalar.dma_start(
                                out=yt[s, 1, :, dd, lo : lo + HN],
                                in_=y_half[:, 0:HN],
                            )
                    else:
                        for k in range(HC):
                            for nt in range(NN):
                                nc.tensor.matmul(
                                    acc2s[nt][:],
                                    w2_t[:, v, k, :],
                                    h_tiles[k][:, nt * NT : (nt + 1) * NT],
                                    start=(k == 0),
                                    stop=(k == HC - 1),
                                )
                        for nt in range(NN):
                            nc.scalar.activation(
                                y_big[nt][:, dd, :],
                                acc2s[nt][:],
                                mybir.ActivationFunctionType.Identity,
                                bias=b_t[:, HC + dd : HC + dd + 1],
                            )
                        if dd == DD_BATCH - 1:
                            for nt in range(NN):
                                nc.scalar.dma_start(
                                    out=yt[s, nt, :, 0:DD_BATCH, :],
                                    in_=y_big[nt][:, 0:DD_BATCH, :],
                                )

    nc.finalize()
    return nc


def _get_nc():
    if "nc" not in _CACHE:
        _CACHE["nc"] = _build_nc()
    return _CACHE["nc"]


def _prep_in_maps(xs, w1, b1, w2, b2):
    xs = np.asarray(xs, dtype=np.float32).astype(NP_BF16)
    w1 = np.asarray(w1, dtype=np.float32).astype(NP_BF16)
    b1 = np.asarray(b1, dtype=np.float32)
    w2 = np.asarray(w2, dtype=np.float32).astype(NP_BF16)
    b2 = np.asarray(b2, dtype=np.float32)

    x3 = xs.reshape(E, NE, D)
    in_maps = []
    for core in range(N_CORES):
        es = [E_LOC * core + s for s in range(E_LOC)]
        # xt[s, nt, p, c, j] = x_e[nt*512+j, c*128+p]
        xt = np.stack(
            [x3[e].reshape(NN, NT, DC, P).transpose(0, 3, 2, 1) for e in es]
        )
        # w1t[s, g, p, u, c, j] = w1[e, (g*4+u)*128+j, c*128+p]
        w1t = np.stack(
            [w1[e].reshape(NG1, W1G, P, DC, P).transpose(0, 4, 1, 3, 2)
             for e in es]
        )
        # w2t[s, g, p, v, k, j] = w2[e, (g*2+v)*128+j, k*128+p]
        w2t = np.stack(
            [w2[e].reshape(NG2, W2G, P, HC, P).transpose(0, 4, 1, 3, 2)
             for e in es]
        )
        # br[s, p, k] = b1[e, k*128+p];  br[s, p, HC+dd] = b2[e, dd*128+p]
        brm = np.stack(
            [np.concatenate(
                [b1[e].reshape(HC, P).T, b2[e].reshape(DC, P).T], axis=1)
             for e in es]
        )
        in_maps.append(
            {
                "xt": np.ascontiguousarray(xt),
                "w1t": np.ascontiguousarray(w1t),
                "w2t": np.ascontiguousarray(w2t),
                "br": np.ascontiguousarray(brm),
            }
        )
    return in_maps


def _gather(results):
    y = np.empty((N_TOK, D), dtype=np.float32)
    for core in range(N_CORES):
        out = results[core]["yt"]  # [E_LOC, NN, P, DC, NT] bf16
        for s in range(E_LOC):
            e = E_LOC * core + s
            # yt[s, nt, p, dd, j] = y_e[nt*512+j, dd*128+p]
            for nt in range(NN):
                y[e * NE + nt * NT : e * NE + (nt + 1) * NT] = (
                    out[s, nt].transpose(2, 1, 0).reshape(NT, D)
                    .astype(np.float32)
                )
    return y


def _run(in_maps, **kwargs):
    nc = _get_nc()
    return run_bass_kernel_spmd(nc, in_maps, core_ids=list(range(N_CORES)), **kwargs)


def kernel(xs, fwd_expert_count, w1, b1, w2, b2):
    # fwd_expert_count is uniform (N_TOK // E per expert) by construction,
    # matching the reference, which also hardcodes the uniform grouping.
    in_maps = _prep_in_maps(xs, w1, b1, w2, b2)
    res = _run(in_maps)
    return _gather(res.results)


# revision 19
# speedup vs baseline: 1.0016x; 1.0012x over previous
"""Expert-parallel grouped-GEMM FFN (MoE expert module) for TRN2, 8 NeuronCores.

Problem: xs [16384, 1024] grouped contiguously into 16 experts x 1024 tokens.
Per expert e: y = relu(x @ w1[e].T + b1[e]) @ w2[e].T + b2[e].

Sharding: expert-parallel, 2 experts per core. Each core computes its two
experts' FFN independently; outputs are disjoint row-blocks of the result, so
no collectives are needed.

Precision: weights and activations are bf16 (host-side cast), accumulation and
biases fp32.  l2 relative error ~3e-3, well inside the 2e-2 gate.

The matmul stream itself runs at the N=512 issue floor (~213 ns/MM warm), so
v2 attacks the only remaining overheads seen in the trace:
  - Startup: the framework preamble blocks all engine queues until ~7.2us, so
    DMA data cannot land before ~10.5us; the old 80-matmul warmup padded the
    PE queue until 13.9us (NX-paced ~91ns each, in-order ahead of the real
    chains).  v2 trims the warmup to ~26 MMs so the first real chain issues
    right as its x/w1 deps land, with the warmup still covering the HAM
    clock-gate ramp (~3.4us of busy to reach 2.4 GHz).
  - Periodic notification stalls: the profile shows a 432 ns PE stall every
    10.79 us - the notification buffer (~640 entries) draining; DMA packets
    (one per partition row) dominate the notification rate.  v2 batches DMAs
    into wider per-partition rows: w1 in 4-k-tile groups (8KB rows), w2 in
    2-dd groups (16KB rows), y per (expert, token-half) (8KB rows), x as one
    2MB tile per expert (16KB rows), b1+b2 combined - cutting packets from
    ~17.6k to ~6k and the stall count proportionally.
  - Tail: the last d-tile runs as two N=256 column-half chains in separate
    PSUM banks so the post-final-matmul drainage covers only 256 columns
    (unchanged from v1).

Per-core schedule (per expert, all matmuls N=512, 128-contraction):
  - mm1: for each of 32 h-tiles k, two 8-matmul PSUM chains (contraction D),
    ACT evicts relu(acc + b1) -> h[k] bf16 in SBUF.  The second token-half
    chain trails DEFER k-tiles so expert 0's first chains need only the first
    token-half of x.
  - mm2: for each of 8 d-tiles, two interleaved 32-matmul PSUM chains
    (contraction H); ACT evicts acc + b2 into a per-token-half y tile that is
    written back in one DMA per (expert, token-half).
  - Startup-critical transfers ride one sync-ring in priority order (w1[0]
    single k-tiles with the x halves behind w1[0,0], then x-nt1, then 4-k
    w1 groups); w2 groups are issued mid-mm1 so the 2MB transfers hide under
    the trailing mm1 chains.
"""

import numpy as np
import ml_dtypes

import concourse.bacc as bacc
import concourse.mybir as mybir
import concourse.tile as tile
from concourse.bass_utils import run_bass_kernel_spmd

P = 128                 # SBUF partitions / PE array dim
D = 1024                # model dim
H = 4096                # hidden dim
E = 16                  # experts
N_TOK = 16384           # total tokens
N_CORES = 8
E_LOC = E // N_CORES    # experts per core = 2
NE = N_TOK // E         # tokens per expert = 1024
DC = D // P             # 8  (d chunks: mm1 contraction / mm2 output)
HC = H // P             # 32 (h chunks)
NT = 512                # matmul moving free dim (one PSUM bank of fp32)
NN = NE // NT           # 2  (token tiles per expert)
DEFER = 6               # k-tiles the second token-half chain trails by
W1G = 4                 # k-tiles per batched w1 DMA (8KB dram rows)
NG1 = HC // W1G         # 8 w1 groups
HEAD = 4                # expert 0's first k-tiles ride as single-k DMAs
W2G = 2                 # dd-tiles per batched w2 DMA (16KB dram rows)
NG2 = DC // W2G         # 4 w2 groups
WARM_MM = 40            # HAM warmup matmuls (cover preamble->first-DMA-landing)

F32 = mybir.dt.float32
BF16 = mybir.dt.bfloat16
NP_BF16 = ml_dtypes.bfloat16

_CACHE = {}


def _build_nc():
    nc = bacc.Bacc(None, target_bir_lowering=False)

    # Host-tiled layouts (see _prep_in_maps for the exact index maps):
    #   xt  [s, nt, p, c, j]      = x_e[nt*512+j, c*128+p]          (8KB rows)
    #   w1t [s, g, p, u, c, j]    = w1[e, (g*4+u)*128+j, c*128+p]   (8KB rows)
    #   w2t [s, g, p, v, k, j]    = w2[e, (g*2+v)*128+j, k*128+p]   (16KB rows)
    #   br  [s, p, 0:HC]          = b1[e, k*128+p]
    #   br  [s, p, HC+dd]         = b2[e, dd*128+p]
    #   yt  [s, nt, p, dd, j]     = y_e[nt*512+j, dd*128+p]         (8KB rows)
    xt = nc.dram_tensor("xt", [E_LOC, NN, P, DC, NT], BF16, kind="ExternalInput")
    # Contiguous duplicates of expert 0's first two w1 k-tiles: the scalar
    # ring's cold first-fetch is much faster on contiguous sources, and these
    # ride the scalar ring in parallel with x-nt0 on the sync ring.
    w1h = nc.dram_tensor("w1h", [2, P, DC, P], BF16, kind="ExternalInput")
    w1t = nc.dram_tensor("w1t", [E_LOC, NG1, P, W1G, DC, P], BF16,
                         kind="ExternalInput")
    w2t = nc.dram_tensor("w2t", [E_LOC, NG2, P, W2G, HC, P], BF16,
                         kind="ExternalInput")
    br = nc.dram_tensor("br", [E_LOC, P, HC + DC], F32, kind="ExternalInput")
    yt = nc.dram_tensor("yt", [E_LOC, NN, P, DC, NT], BF16, kind="ExternalOutput")

    with tile.TileContext(nc) as tc:
        with (
            tc.tile_pool(name="xpool", bufs=NN) as xpool,
            tc.tile_pool(name="hpool", bufs=HC + 4) as hpool,
            tc.tile_pool(name="w1hpool", bufs=HEAD) as w1hpool,
            tc.tile_pool(name="w1pool", bufs=4) as w1pool,
            tc.tile_pool(name="w2pool", bufs=2) as w2pool,
            tc.tile_pool(name="ybpool", bufs=NN) as ybpool,
            tc.tile_pool(name="ypool", bufs=4) as ypool,
            tc.tile_pool(name="cpool", bufs=E_LOC) as cpool,
            tc.tile_pool(name="wpool", bufs=1) as wpool,
            tc.tile_pool(name="ps1", bufs=4, space="PSUM") as ps1,
            tc.tile_pool(name="ps2", bufs=4, space="PSUM") as ps2,
        ):
            # HAM warm-up: throwaway matmuls on a zeroed scratch tile keep the
            # PE busy from preamble-exit (~7.2us) until the first chain's x/w1
            # DMAs land (~10.5us), so the clock gate reaches 8/8 (2.4 GHz)
            # before the real chains start.  These are in-order ahead of the
            # real chains (NX-paced ~107ns each cold), so the count must NOT
            # overshoot the DMA landing time.
            warm = wpool.tile([P, P], BF16)
            nc.gpsimd.memset(warm[:], 0.0)
            warm_acc = ps1.tile([P, NT], F32, name="acc", tag="acc")
            for _ in range(WARM_MM):
                nc.tensor.matmul(warm_acc[:, 0:P], warm[:], warm[:],
                                 start=True, stop=True)

            # Expert 0's first x token-half rides the sync ring right behind
            # the first w1 k-tile: HBM is the startup bottleneck, and the
            # first chain's deps must complete earliest.  Tile dependencies
            # are tracked per tile (a reader waits for ALL of a tile's
            # writers), so every startup-critical transfer gets its own tile:
            # per-k w1 head tiles and per-token-half x tiles.
            x_cur = [xpool.tile([P, DC, NT], BF16, name="x_t", tag="x_t")
                     for _ in range(NN)]

            for s in range(E_LOC):
                # ---------------- mm1: h = relu(x @ w1.T + b1) ----------------
                h_tiles = []
                w1h_tiles = []
                w1g_tiles = {}
                w2g_tiles = []
                n_head = HEAD if s == 0 else 0

                def load_w1_group(g, s=s):
                    t = w1pool.tile([P, W1G, DC, P], BF16, name="w1g", tag="w1g")
                    nc.sync.dma_start(out=t[:], in_=w1t[s, g])
                    w1g_tiles[g] = t

                def w1_slice(k, c, s=s, n_head=n_head):
                    if k < n_head:
                        return w1h_tiles[k][:, c, :]
                    return w1g_tiles[k // W1G][:, k % W1G, c, :]

                def mm1_chain(k, nt, s=s):
                    acc = ps1.tile([P, NT], F32, name="acc", tag="acc")
                    for c in range(DC):
                        nc.tensor.matmul(
                            acc[:],
                            w1_slice(k, c),
                            x_cur[nt][:, c, :],
                            start=(c == 0),
                            stop=(c == DC - 1),
                        )
                    nc.scalar.activation(
                        h_tiles[k][:, nt * NT : (nt + 1) * NT],
                        acc[:],
                        mybir.ActivationFunctionType.Relu,
                        bias=b_t[:, k : k + 1],
                    )

                if s == 0:
                    # Startup is DMA-landing-bound.  Per-queue rate is
                    # row-size-dependent (2KB rows ~175 GB/s, 8KB rows
                    # ~390 GB/s), so x-nt0 rides the sync ring as one
                    # 8KB-row transfer while w1[0]/w1[1] (contiguous dram
                    # copies) ride the scalar ring in parallel.
                    # First chain (needs w1[0] + x-nt0) is ready ~12us.
                    w1h_tiles = [w1hpool.tile([P, DC, P], BF16,
                                              name="w1h", tag="w1h")
                                 for _ in range(HEAD)]
                    nc.scalar.dma_start(out=w1h_tiles[0][:], in_=w1h[0])
                    nc.sync.dma_start(out=x_cur[0][:], in_=xt[0, 0])
                    nc.scalar.dma_start(out=w1h_tiles[1][:], in_=w1h[1])
                    for k in range(2, HEAD):
                        nc.sync.dma_start(out=w1h_tiles[k][:],
                                          in_=w1t[0, k // W1G, :, k % W1G])
                    b_t = cpool.tile([P, HC + DC], F32)
                    nc.scalar.dma_start(out=b_t[:], in_=br[s])
                    load_w1_group(1)
                    nc.sync.dma_start(out=x_cur[1][:], in_=xt[0, 1])
                else:
                    b_t = cpool.tile([P, HC + DC], F32)
                    nc.scalar.dma_start(out=b_t[:], in_=br[s])
                    load_w1_group(0)
                    load_w1_group(1)

                for k in range(HC):
                    # Keep ~2 w1 groups of lookahead on the sync ring.
                    if k % W1G == 0:
                        g = k // W1G + 2
                        if g < NG1 and (g >= n_head // W1G):
                            load_w1_group(g)
                    # w2 group DMAs (2MB each) issue mid-mm1 so they hide
                    # under the trailing mm1 chains.
                    if k == 20 or k == 28:
                        t = w2pool.tile([P, W2G, HC, P], BF16,
                                        name="w2g", tag="w2g")
                        nc.sync.dma_start(out=t[:], in_=w2t[s, len(w2g_tiles)])
                        w2g_tiles.append(t)
                    h_tiles.append(hpool.tile([P, NE], BF16, name="h_t", tag="h_t"))
                    mm1_chain(k, 0)
                    if k >= DEFER:
                        mm1_chain(k - DEFER, 1)
                for k in range(HC - DEFER, HC):
                    mm1_chain(k, 1)

                # Prefetch next expert's x while this expert's mm2 runs
                # (xpool bufs=NN defers it until mm1(s) retires).
                if s + 1 < E_LOC:
                    x_next = [xpool.tile([P, DC, NT], BF16,
                                         name="x_t", tag="x_t")
                              for _ in range(NN)]
                    for nt in range(NN):
                        nc.scalar.dma_start(out=x_next[nt][:],
                                            in_=xt[s + 1, nt])
                    x_cur = x_next

                # ---------------- mm2: y = h @ w2.T + b2 ----------------
                # Evictions land in one bf16 y tile per token-half; each tile
                # is written back in a single 8KB-row DMA.  For the last
                # expert the final d-tile stays fine-grained (N=256 halves)
                # to keep the kernel tail short.
                y_big = [ybpool.tile([P, DC, NT], BF16, name="y_b", tag="y_b")
                         for _ in range(NN)]
                DD_BATCH = DC - 1 if s == E_LOC - 1 else DC

                for dd in range(DC):
                    if dd >= len(w2g_tiles) * W2G:
                        t = w2pool.tile([P, W2G, HC, P], BF16,
                                        name="w2g", tag="w2g")
                        nc.sync.dma_start(out=t[:], in_=w2t[s, len(w2g_tiles)])
                        w2g_tiles.append(t)
                    w2_t = w2g_tiles[dd // W2G]
                    v = dd % W2G
                    last = s == E_LOC - 1 and dd == DC - 1
                    acc2s = [ps2.tile([P, NT], F32, name="acc2", tag="acc2")
                             for _ in range(NN)]

                    if last:
                        # nt0: plain chain; its eviction + writeback hide under
                        # nt1's work.  nt1 runs as two N=256 column-half chains
                        # in separate PSUM banks: the first half's eviction and
                        # writeback hide under the second half's ~3.5us chain,
                        # and the post-final-matmul drainage (evict + DMA +
                        # completion receipt) covers only 256 columns.
                        for k in range(HC):
                            nc.tensor.matmul(
                                acc2s[0][:],
                                w2_t[:, v, k, :],
                                h_tiles[k][:, 0:NT],
                                start=(k == 0),
                                stop=(k == HC - 1),
                            )
                        y_tile = ypool.tile([P, NT], BF16)
                        nc.scalar.activation(
                            y_tile[:],
                            acc2s[0][:],
                            mybir.ActivationFunctionType.Identity,
                            bias=b_t[:, HC + dd : HC + dd + 1],
                        )
                        nc.scalar.dma_start(
                            out=yt[s, 0, :, dd, :],
                            in_=y_tile[:],
                        )
                        HN = NT // 2
                        accR = ps2.tile([P, NT], F32, name="acc2", tag="acc2")
                        for half, acch in ((0, acc2s[1]), (1, accR)):
                            lo = half * HN
                            for k in range(HC):
                                nc.tensor.matmul(
                                    acch[:, 0:HN],
                                    w2_t[:, v, k, :],
                                    h_tiles[k][:, NT + lo : NT + lo + HN],
                                    start=(k == 0),
                                    stop=(k == HC - 1),
                                )
                            y_half = ypool.tile([P, NT], BF16)
                            nc.scalar.activation(
                                y_half[:, 0:HN],
                                acch[:, 0:HN],
                                mybir.ActivationFunctionType.Identity,
                                bias=b_t[:, HC + dd : HC + dd + 1],
                            )
                            nc.scalar.dma_start(
                                out=yt[s, 1, :, dd, lo : lo + HN],
                                in_=y_half[:, 0:HN],
                            )
                    else:
                        for k in range(HC):
                            for nt in range(NN):
                                nc.tensor.matmul(
                                    acc2s[nt][:],
                                    w2_t[:, v, k, :],
                                    h_tiles[k][:, nt * NT : (nt + 1) * NT],
                                    start=(k == 0),
                                    stop=(k == HC - 1),
                                )
                        for nt in range(NN):
                            nc.scalar.activation(
                                y_big[nt][:, dd, :],
                                acc2s[nt][:],
                                mybir.ActivationFunctionType.Identity,
                                bias=b_t[:, HC + dd : HC + dd + 1],
                            )
                        if dd == DD_BATCH - 1:
                            for nt in range(NN):
                                nc.scalar.dma_start(
                                    out=yt[s, nt, :, 0:DD_BATCH, :],
                                    in_=y_big[nt][:, 0:DD_BATCH, :],
                                )

    nc.finalize()
    return nc


def _get_nc():
    if "nc" not in _CACHE:
        _CACHE["nc"] = _build_nc()
    return _CACHE["nc"]


def _prep_in_maps(xs, w1, b1, w2, b2):
    xs = np.asarray(xs, dtype=np.float32).astype(NP_BF16)
    w1 = np.asarray(w1, dtype=np.float32).astype(NP_BF16)
    b1 = np.asarray(b1, dtype=np.float32)
    w2 = np.asarray(w2, dtype=np.float32).astype(NP_BF16)
    b2 = np.asarray(b2, dtype=np.float32)

    x3 = xs.reshape(E, NE, D)
    in_maps = []
    for core in range(N_CORES):
        es = [E_LOC * core + s for s in range(E_LOC)]
        # xt[s, nt, p, c, j] = x_e[nt*512+j, c*128+p]
        xt = np.stack(
            [x3[e].reshape(NN, NT, DC, P).transpose(0, 3, 2, 1) for e in es]
        )
        # w1t[s, g, p, u, c, j] = w1[e, (g*4+u)*128+j, c*128+p]
        w1t = np.stack(
            [w1[e].reshape(NG1, W1G, P, DC, P).transpose(0, 4, 1, 3, 2)
             for e in es]
        )
        # w1h[k, p, c, j] = w1[e0, k*128+j, c*128+p] for k in {0, 1}
        w1hm = w1[es[0]][: 2 * P].reshape(2, P, DC, P).transpose(0, 3, 2, 1)
        # w2t[s, g, p, v, k, j] = w2[e, (g*2+v)*128+j, k*128+p]
        w2t = np.stack(
            [w2[e].reshape(NG2, W2G, P, HC, P).transpose(0, 4, 1, 3, 2)
             for e in es]
        )
        # br[s, p, k] = b1[e, k*128+p];  br[s, p, HC+dd] = b2[e, dd*128+p]
        brm = np.stack(
            [np.concatenate(
                [b1[e].reshape(HC, P).T, b2[e].reshape(DC, P).T], axis=1)
             for e in es]
        )
        in_maps.append(
            {
                "xt": np.ascontiguousarray(xt),
                "w1h": np.ascontiguousarray(w1hm),
                "w1t": np.ascontiguousarray(w1t),
                "w2t": np.ascontiguousarray(w2t),
                "br": np.ascontiguousarray(brm),
            }
        )
    return in_maps


def _gather(results):
    y = np.empty((N_TOK, D), dtype=np.float32)
    for core in range(N_CORES):
        out = results[core]["yt"]  # [E_LOC, NN, P, DC, NT] bf16
        for s in range(E_LOC):
            e = E_LOC * core + s
            # yt[s, nt, p, dd, j] = y_e[nt*512+j, dd*128+p]
            for nt in range(NN):
                y[e * NE + nt * NT : e * NE + (nt + 1) * NT] = (
                    out[s, nt].transpose(2, 1, 0).reshape(NT, D)
                    .astype(np.float32)
                )
    return y


def _run(in_maps, **kwargs):
    nc = _get_nc()
    return run_bass_kernel_spmd(nc, in_maps, core_ids=list(range(N_CORES)), **kwargs)


def kernel(xs, fwd_expert_count, w1, b1, w2, b2):
    # fwd_expert_count is uniform (N_TOK // E per expert) by construction,
    # matching the reference, which also hardcodes the uniform grouping.
    in_maps = _prep_in_maps(xs, w1, b1, w2, b2)
    res = _run(in_maps)
    return _gather(res.results)


# revision 25
# speedup vs baseline: 1.0069x; 1.0053x over previous
"""Expert-parallel grouped-GEMM FFN (MoE expert module) for TRN2, 8 NeuronCores.

Problem: xs [16384, 1024] grouped contiguously into 16 experts x 1024 tokens.
Per expert e: y = relu(x @ w1[e].T + b1[e]) @ w2[e].T + b2[e].

Sharding: expert-parallel, 2 experts per core. Each core computes its two
experts' FFN independently; outputs are disjoint row-blocks of the result, so
no collectives are needed.

Precision: weights and activations are bf16 (host-side cast), accumulation and
biases fp32.  l2 relative error ~3e-3, well inside the 2e-2 gate.

The matmul stream itself runs at the N=512 issue floor (~213 ns/MM warm), so
v2 attacks the only remaining overheads seen in the trace:
  - Startup: the framework preamble blocks all engine queues until ~7.2us, so
    DMA data cannot land before ~10.5us; the old 80-matmul warmup padded the
    PE queue until 13.9us (NX-paced ~91ns each, in-order ahead of the real
    chains).  v2 trims the warmup to ~26 MMs so the first real chain issues
    right as its x/w1 deps land, with the warmup still covering the HAM
    clock-gate ramp (~3.4us of busy to reach 2.4 GHz).
  - Periodic notification stalls: the profile shows a 432 ns PE stall every
    10.79 us - the notification buffer (~640 entries) draining; DMA packets
    (one per partition row) dominate the notification rate.  v2 batches DMAs
    into wider per-partition rows: w1 in 4-k-tile groups (8KB rows), w2 in
    2-dd groups (16KB rows), y per (expert, token-half) (8KB rows), x as one
    2MB tile per expert (16KB rows), b1+b2 combined - cutting packets from
    ~17.6k to ~6k and the stall count proportionally.
  - Tail: the last d-tile runs as two N=256 column-half chains in separate
    PSUM banks so the post-final-matmul drainage covers only 256 columns
    (unchanged from v1).

Per-core schedule (per expert, all matmuls N=512, 128-contraction):
  - mm1: for each of 32 h-tiles k, two 8-matmul PSUM chains (contraction D),
    ACT evicts relu(acc + b1) -> h[k] bf16 in SBUF.  The second token-half
    chain trails DEFER k-tiles so expert 0's first chains need only the first
    token-half of x.
  - mm2: for each of 8 d-tiles, two interleaved 32-matmul PSUM chains
    (contraction H); ACT evicts acc + b2 into a per-token-half y tile that is
    written back in one DMA per (expert, token-half).
  - Startup-critical transfers ride one sync-ring in priority order (w1[0]
    single k-tiles with the x halves behind w1[0,0], then x-nt1, then 4-k
    w1 groups); w2 groups are issued mid-mm1 so the 2MB transfers hide under
    the trailing mm1 chains.
"""

import numpy as np
import ml_dtypes

import concourse.bacc as bacc
import concourse.mybir as mybir
import concourse.tile as tile
from concourse.bass_utils import run_bass_kernel_spmd

P = 128                 # SBUF partitions / PE array dim
D = 1024                # model dim
H = 4096                # hidden dim
E = 16                  # experts
N_TOK = 16384           # total tokens
N_CORES = 8
E_LOC = E // N_CORES    # experts per core = 2
NE = N_TOK // E         # tokens per expert = 1024
DC = D // P             # 8  (d chunks: mm1 contraction / mm2 output)
HC = H // P             # 32 (h chunks)
NT = 512                # matmul moving free dim (one PSUM bank of fp32)
NN = NE // NT           # 2  (token tiles per expert)
DEFER = 6               # k-tiles the second token-half chain trails by
W1G = 4                 # k-tiles per batched w1 DMA (8KB dram rows)
NG1 = HC // W1G         # 8 w1 groups
HEAD = 8                # expert 0's first k-tiles ride as single-k DMAs
W2G = 2                 # dd-tiles per batched w2 DMA (16KB dram rows)
NG2 = DC // W2G         # 4 w2 groups
WARM_MM = 80            # HAM warmup matmuls (cover preamble->first-DMA-landing)

F32 = mybir.dt.float32
BF16 = mybir.dt.bfloat16
NP_BF16 = ml_dtypes.bfloat16

_CACHE = {}


def _build_nc():
    nc = bacc.Bacc(None, target_bir_lowering=False)

    # Host-tiled layouts (see _prep_in_maps for the exact index maps):
    #   xt  [s, nt, p, c, j]      = x_e[nt*512+j, c*128+p]          (8KB rows)
    #   w1t [s, g, p, u, c, j]    = w1[e, (g*4+u)*128+j, c*128+p]   (8KB rows)
    #   w2t [s, g, p, v, k, j]    = w2[e, (g*2+v)*128+j, k*128+p]   (16KB rows)
    #   br  [s, p, 0:HC]          = b1[e, k*128+p]
    #   br  [s, p, HC+dd]         = b2[e, dd*128+p]
    #   yt  [s, nt, p, dd, j]     = y_e[nt*512+j, dd*128+p]         (8KB rows)
    xt = nc.dram_tensor("xt", [E_LOC, NN, P, DC, NT], BF16, kind="ExternalInput")
    # Contiguous duplicates of expert 0's first HEAD w1 k-tiles: the startup-
    # critical stream rides the sync ring as per-k single-tile transfers
    # (every other ring's DGE lags ~3us on its cold first fetch).
    w1h = nc.dram_tensor("w1h", [HEAD, P, DC, P], BF16, kind="ExternalInput")
    w1t = nc.dram_tensor("w1t", [E_LOC, NG1, P, W1G, DC, P], BF16,
                         kind="ExternalInput")
    w2t = nc.dram_tensor("w2t", [E_LOC, NG2, P, W2G, HC, P], BF16,
                         kind="ExternalInput")
    br = nc.dram_tensor("br", [E_LOC, P, HC + DC], F32, kind="ExternalInput")
    yt = nc.dram_tensor("yt", [E_LOC, NN, P, DC, NT], BF16, kind="ExternalOutput")

    with tile.TileContext(nc) as tc:
        with (
            tc.tile_pool(name="xpool", bufs=NN) as xpool,
            tc.tile_pool(name="hpool", bufs=HC + 4) as hpool,
            tc.tile_pool(name="w1hpool", bufs=HEAD) as w1hpool,
            tc.tile_pool(name="w1pool", bufs=4) as w1pool,
            tc.tile_pool(name="w2pool", bufs=2) as w2pool,
            tc.tile_pool(name="ybpool", bufs=NN) as ybpool,
            tc.tile_pool(name="ypool", bufs=4) as ypool,
            tc.tile_pool(name="cpool", bufs=E_LOC) as cpool,
            tc.tile_pool(name="wpool", bufs=1) as wpool,
            tc.tile_pool(name="ps1", bufs=4, space="PSUM") as ps1,
            tc.tile_pool(name="ps2", bufs=4, space="PSUM") as ps2,
        ):
            # HAM warm-up: throwaway matmuls on a zeroed scratch tile keep the
            # PE busy from preamble-exit (~7.2us) until the first chain's x/w1
            # DMAs land (~10.5us), so the clock gate reaches 8/8 (2.4 GHz)
            # before the real chains start.  These are in-order ahead of the
            # real chains (NX-paced ~107ns each cold), so the count must NOT
            # overshoot the DMA landing time.
            warm = wpool.tile([P, P], BF16)
            nc.gpsimd.memset(warm[:], 0.0)
            warm_acc = ps1.tile([P, NT], F32, name="acc", tag="acc")
            for _ in range(WARM_MM):
                nc.tensor.matmul(warm_acc[:, 0:P], warm[:], warm[:],
                                 start=True, stop=True)

            # Expert 0's first x token-half rides the sync ring right behind
            # the first w1 k-tile: HBM is the startup bottleneck, and the
            # first chain's deps must complete earliest.  Tile dependencies
            # are tracked per tile (a reader waits for ALL of a tile's
            # writers), so every startup-critical transfer gets its own tile:
            # per-k w1 head tiles and per-token-half x tiles.
            x_cur = [xpool.tile([P, DC, NT], BF16, name="x_t", tag="x_t")
                     for _ in range(NN)]

            for s in range(E_LOC):
                # ---------------- mm1: h = relu(x @ w1.T + b1) ----------------
                h_tiles = []
                w1h_tiles = []
                w1g_tiles = {}
                w2g_tiles = []
                n_head = HEAD if s == 0 else 0

                def load_w1_group(g, s=s):
                    t = w1pool.tile([P, W1G, DC, P], BF16, name="w1g", tag="w1g")
                    nc.sync.dma_start(out=t[:], in_=w1t[s, g])
                    w1g_tiles[g] = t

                def w1_slice(k, c, s=s, n_head=n_head):
                    if k < n_head:
                        return w1h_tiles[k][:, c, :]
                    return w1g_tiles[k // W1G][:, k % W1G, c, :]

                def mm1_chain(k, nt, s=s):
                    acc = ps1.tile([P, NT], F32, name="acc", tag="acc")
                    for c in range(DC):
                        nc.tensor.matmul(
                            acc[:],
                            w1_slice(k, c),
                            x_cur[nt][:, c, :],
                            start=(c == 0),
                            stop=(c == DC - 1),
                        )
                    nc.scalar.activation(
                        h_tiles[k][:, nt * NT : (nt + 1) * NT],
                        acc[:],
                        mybir.ActivationFunctionType.Relu,
                        bias=b_t[:, k : k + 1],
                    )

                if s == 0:
                    # Startup-critical transfers ride one sync ring in
                    # priority order (w1[0], x-nt0 halves, w1[1..7], x-nt1):
                    # HBM is the startup bottleneck, the first-chain deps
                    # must complete earliest, and their completion receipts
                    # pipeline with the trailing MMs of each chain.
                    b_t = cpool.tile([P, HC + DC], F32)
                    nc.scalar.dma_start(out=b_t[:], in_=br[s])
                    for k in range(HEAD):
                        t = w1hpool.tile([P, DC, P], BF16,
                                         name="w1h", tag="w1h")
                        nc.sync.dma_start(out=t[:], in_=w1h[k])
                        w1h_tiles.append(t)
                        if k == 0:
                            nc.sync.dma_start(out=x_cur[0][:, 0 : DC // 2, :],
                                              in_=xt[0, 0, :, 0 : DC // 2, :])
                            nc.sync.dma_start(out=x_cur[0][:, DC // 2 :, :],
                                              in_=xt[0, 0, :, DC // 2 :, :])
                    nc.sync.dma_start(out=x_cur[1][:], in_=xt[0, 1])
                else:
                    b_t = cpool.tile([P, HC + DC], F32)
                    nc.scalar.dma_start(out=b_t[:], in_=br[s])
                    load_w1_group(0)
                    load_w1_group(1)

                for k in range(HC):
                    # Keep ~2 w1 groups of lookahead on the sync ring.
                    if k % W1G == 0:
                        g = k // W1G + 2
                        if g < NG1 and (g >= n_head // W1G):
                            load_w1_group(g)
                    # w2 group DMAs (2MB each) issue mid-mm1 so they hide
                    # under the trailing mm1 chains.
                    if k == 20 or k == 28:
                        t = w2pool.tile([P, W2G, HC, P], BF16,
                                        name="w2g", tag="w2g")
                        nc.sync.dma_start(out=t[:], in_=w2t[s, len(w2g_tiles)])
                        w2g_tiles.append(t)
                    h_tiles.append(hpool.tile([P, NE], BF16, name="h_t", tag="h_t"))
                    mm1_chain(k, 0)
                    if k >= DEFER:
                        mm1_chain(k - DEFER, 1)
                for k in range(HC - DEFER, HC):
                    mm1_chain(k, 1)

                # Prefetch next expert's x while this expert's mm2 runs
                # (xpool bufs=NN defers it until mm1(s) retires).
                if s + 1 < E_LOC:
                    x_next = [xpool.tile([P, DC, NT], BF16,
                                         name="x_t", tag="x_t")
                              for _ in range(NN)]
                    for nt in range(NN):
                        nc.scalar.dma_start(out=x_next[nt][:],
                                            in_=xt[s + 1, nt])
                    x_cur = x_next

                # ---------------- mm2: y = h @ w2.T + b2 ----------------
                # Evictions land in one bf16 y tile per token-half; each tile
                # is written back in a single 8KB-row DMA.  For the last
                # expert the final d-tile stays fine-grained (N=256 halves)
                # to keep the kernel tail short.
                y_big = [ybpool.tile([P, DC, NT], BF16, name="y_b", tag="y_b")
                         for _ in range(NN)]
                DD_BATCH = DC - 1 if s == E_LOC - 1 else DC

                for dd in range(DC):
                    if dd >= len(w2g_tiles) * W2G:
                        t = w2pool.tile([P, W2G, HC, P], BF16,
                                        name="w2g", tag="w2g")
                        nc.sync.dma_start(out=t[:], in_=w2t[s, len(w2g_tiles)])
                        w2g_tiles.append(t)
                    w2_t = w2g_tiles[dd // W2G]
                    v = dd % W2G
                    last = s == E_LOC - 1 and dd == DC - 1
                    acc2s = [ps2.tile([P, NT], F32, name="acc2", tag="acc2")
                             for _ in range(NN)]

                    if last:
                        # nt0: plain chain; its eviction + writeback hide under
                        # nt1's work.  nt1 runs as two N=256 column-half chains
                        # in separate PSUM banks: the first half's eviction and
                        # writeback hide under the second half's ~3.5us chain,
                        # and the post-final-matmul drainage (evict + DMA +
                        # completion receipt) covers only 256 columns.
                        for k in range(HC):
                            nc.tensor.matmul(
                                acc2s[0][:],
                                w2_t[:, v, k, :],
                                h_tiles[k][:, 0:NT],
                                start=(k == 0),
                                stop=(k == HC - 1),
                            )
                        y_tile = ypool.tile([P, NT], BF16)
                        nc.scalar.activation(
                            y_tile[:],
                            acc2s[0][:],
                            mybir.ActivationFunctionType.Identity,
                            bias=b_t[:, HC + dd : HC + dd + 1],
                        )
                        nc.scalar.dma_start(
                            out=yt[s, 0, :, dd, :],
                            in_=y_tile[:],
                        )
                        # Uneven split (384, 128): the first part's eviction
                        # and writeback hide under the second chain, and the
                        # post-final-matmul drainage covers only 128 columns.
                        HN0 = 384
                        accR = ps2.tile([P, NT], F32, name="acc2", tag="acc2")
                        for lo, hn, acch in ((0, HN0, acc2s[1]),
                                             (HN0, NT - HN0, accR)):
                            for k in range(HC):
                                nc.tensor.matmul(
                                    acch[:, 0:hn],
                                    w2_t[:, v, k, :],
                                    h_tiles[k][:, NT + lo : NT + lo + hn],
                                    start=(k == 0),
                                    stop=(k == HC - 1),
                                )
                            y_half = ypool.tile([P, NT], BF16)
                            nc.scalar.activation(
                                y_half[:, 0:hn],
                                acch[:, 0:hn],
                                mybir.ActivationFunctionType.Identity,
                                bias=b_t[:, HC + dd : HC + dd + 1],
                            )
                            nc.scalar.dma_start(
                                out=yt[s, 1, :, dd, lo : lo + hn],
                                in_=y_half[:, 0:hn],
                            )
                    else:
                        for k in range(HC):
                            for nt in range(NN):
                                nc.tensor.matmul(
                                    acc2s[nt][:],
                                    w2_t[:, v, k, :],
                                    h_tiles[k][:, nt * NT : (nt + 1) * NT],
                                    start=(k == 0),
                                    stop=(k == HC - 1),
                                )
                        for nt in range(NN):
                            nc.scalar.activation(
                                y_big[nt][:, dd, :],
                                acc2s[nt][:],
                                mybir.ActivationFunctionType.Identity,
                                bias=b_t[:, HC + dd : HC + dd + 1],
                            )
                        if dd == DD_BATCH - 1:
                            for nt in range(NN):
                                nc.scalar.dma_start(
                                    out=yt[s, nt, :, 0:DD_BATCH, :],
                                    in_=y_big[nt][:, 0:DD_BATCH, :],
                                )

    nc.finalize()
    return nc


def _get_nc():
    if "nc" not in _CACHE:
        _CACHE["nc"] = _build_nc()
    return _CACHE["nc"]


def _prep_in_maps(xs, w1, b1, w2, b2):
    xs = np.asarray(xs, dtype=np.float32).astype(NP_BF16)
    w1 = np.asarray(w1, dtype=np.float32).astype(NP_BF16)
    b1 = np.asarray(b1, dtype=np.float32)
    w2 = np.asarray(w2, dtype=np.float32).astype(NP_BF16)
    b2 = np.asarray(b2, dtype=np.float32)

    x3 = xs.reshape(E, NE, D)
    in_maps = []
    for core in range(N_CORES):
        es = [E_LOC * core + s for s in range(E_LOC)]
        # xt[s, nt, p, c, j] = x_e[nt*512+j, c*128+p]
        xt = np.stack(
            [x3[e].reshape(NN, NT, DC, P).transpose(0, 3, 2, 1) for e in es]
        )
        # w1t[s, g, p, u, c, j] = w1[e, (g*4+u)*128+j, c*128+p]
        w1t = np.stack(
            [w1[e].reshape(NG1, W1G, P, DC, P).transpose(0, 4, 1, 3, 2)
             for e in es]
        )
        # w1h[k, p, c, j] = w1[e0, k*128+j, c*128+p] for k < HEAD
        w1hm = (w1[es[0]][: HEAD * P].reshape(HEAD, P, DC, P)
                .transpose(0, 3, 2, 1))
        # w2t[s, g, p, v, k, j] = w2[e, (g*2+v)*128+j, k*128+p]
        w2t = np.stack(
            [w2[e].reshape(NG2, W2G, P, HC, P).transpose(0, 4, 1, 3, 2)
             for e in es]
        )
        # br[s, p, k] = b1[e, k*128+p];  br[s, p, HC+dd] = b2[e, dd*128+p]
        brm = np.stack(
            [np.concatenate(
                [b1[e].reshape(HC, P).T, b2[e].reshape(DC, P).T], axis=1)
             for e in es]
        )
        in_maps.append(
            {
                "xt": np.ascontiguousarray(xt),
                "w1h": np.ascontiguousarray(w1hm),
                "w1t": np.ascontiguousarray(w1t),
                "w2t": np.ascontiguousarray(w2t),
                "br": np.ascontiguousarray(brm),
            }
        )
    return in_maps


def _gather(results):
    y = np.empty((N_TOK, D), dtype=np.float32)
    for core in range(N_CORES):
        out = results[core]["yt"]  # [E_LOC, NN, P, DC, NT] bf16
        for s in range(E_LOC):
            e = E_LOC * core + s
            # yt[s, nt, p, dd, j] = y_e[nt*512+j, dd*128+p]
            for nt in range(NN):
                y[e * NE + nt * NT : e * NE + (nt + 1) * NT] = (
                    out[s, nt].transpose(2, 1, 0).reshape(NT, D)
                    .astype(np.float32)
                )
    return y


def _run(in_maps, **kwargs):
    nc = _get_nc()
    return run_bass_kernel_spmd(nc, in_maps, core_ids=list(range(N_CORES)), **kwargs)


def kernel(xs, fwd_expert_count, w1, b1, w2, b2):
    # fwd_expert_count is uniform (N_TOK // E per expert) by construction,
    # matching the reference, which also hardcodes the uniform grouping.
    in_maps = _prep_in_maps(xs, w1, b1, w2, b2)
    res = _run(in_maps)
    return _gather(res.results)
